# revision 15
# baseline (speedup 1.0000x reference)
"""DividedAttention (TimeSformer-style divided space-time attention) on 8 trn2 cores.

Sharding: pure data-parallel over batch B=16 -> 2 batch items per core.

v2: two-batch software pipeline. The per-core schedule interleaves batch 1's
QKV projection into batch 0's attention steps and batch 0's out-projection
into batch 1's attention steps, so the PE never idles long enough for the
HAM clock gate to re-throttle it to 1.2 GHz (the dominant cost in v1).
Other changes vs v1:
  - one fused exp per attention step (st psum tile [128,2par,2chunk,256],
    single ACTIVATE over all 4 S^T quadrants)
  - PSUM re-plan: proj 2 banks + st 2x2 banks + po 2x1 bank = 8
  - AV output and column sums share one psum bank (cols 0:197 / 256:453)
  - bias + psum->sbuf staging fused in one vector tensor_tensor; the three
    chain-bound final tiles instead use scalar-engine copy with the bias
    applied by a K=1 ones-row matmul, so the vector engine is not the
    serial tail
  - exp(S^T) results live in a single 4-slot ring tile indexed by step,
    enabling frame-major step order for batch 1 (out-projection tiles
    start as frames complete)
  - xT loads split across the scalar and sync hwdge queues
"""
import sys

sys.path.insert(0, "/opt/trn_rl_repo")

import numpy as np
import ml_dtypes

from concourse import bacc
import concourse.mybir as mybir
import concourse.tile as tile
from concourse import bass_utils

BF16 = mybir.dt.bfloat16
F32 = mybir.dt.float32
NPBF = ml_dtypes.bfloat16

B, SP, F, DIM, H, DH = 16, 196, 8, 512, 8, 64
INNER = H * DH            # 512
N = 1 + F * SP            # 1569
SP1 = SP + 1              # 197
NCORES = 8
NB = B // NCORES          # 2
KC = DIM // 128           # 4
NT = (N + 127) // 128     # 13
LAST = N - 128 * (NT - 1)  # 33
TCH = [(0, 1 + 2 * SP), (1 + 2 * SP, 2 * SP), (1 + 4 * SP, 2 * SP), (1 + 6 * SP, 2 * SP)]

EXP = mybir.ActivationFunctionType.Exp
ADD = mybir.AluOpType.add
MULT = mybir.AluOpType.mult

LAG = 2
EARING = 4
NSTEP = NB * 32           # 64 attention steps, hp-major per batch


def _v_pieces(tok0, length):
    out = []
    done = 0
    while done < length:
        tok = tok0 + done
        t, p0 = divmod(tok, 128)
        l = min(128 - p0, length - done)
        out.append((t, p0, done, l))
        done += l
    return out


def _rearr_by_tile():
    """v_flat tile -> list of (dst_kind, frame, src_p0, dst_p0, len)."""
    by_tile = {}
    for f in range(F):
        for (t, p0, d0, l) in _v_pieces(1 + SP * f, 127):
            by_tile.setdefault(t, []).append(("a", f, p0, 1 + d0, l))
        for (t, p0, d0, l) in _v_pieces(128 + SP * f, 69):
            by_tile.setdefault(t, []).append(("b", f, p0, d0, l))
    return by_tile


def build_nc():
    nc = bacc.Bacc(num_devices=NCORES)

    xT = nc.declare_dram_parameter("xT", [NB, DIM, N], BF16, isOutput=False)
    wqkv = nc.declare_dram_parameter("wqkv", [DIM, 3 * INNER], BF16, isOutput=False)
    wout = nc.declare_dram_parameter("wout", [INNER, DIM], BF16, isOutput=False)
    bout = nc.declare_dram_parameter("bout", [1, DIM], F32, isOutput=False)
    out = nc.declare_dram_parameter("out", [NB, N, DIM], F32, isOutput=True)

    rearr = _rearr_by_tile()

    with tile.TileContext(nc) as tc:
        with (
            tc.tile_pool(name="const", bufs=1) as const,
            tc.tile_pool(name="perb", bufs=2) as perb,
            tc.tile_pool(name="vflat", bufs=1) as vflat_pool,
            tc.tile_pool(name="clsp", bufs=2) as clsp,
            tc.tile_pool(name="small", bufs=3) as small,
            tc.tile_pool(name="outp", bufs=3) as outp,
            tc.tile_pool(name="ps_proj", bufs=2, space="PSUM") as ps_proj,
            tc.tile_pool(name="ps_st", bufs=2, space="PSUM") as ps_st,
            tc.tile_pool(name="ps_po", bufs=2, space="PSUM") as ps_po,
        ):
            # ---- constants
            wqkv_sb = const.tile([128, KC, 3 * INNER], BF16)
            for kc in range(KC):
                nc.sync.dma_start(out=wqkv_sb[:, kc, 2 * INNER:3 * INNER],
                                  in_=wqkv[128 * kc:128 * (kc + 1), 2 * INNER:3 * INNER])
            wout_sb = const.tile([128, KC, DIM], BF16)
            ones128 = const.tile([128, 64], BF16)
            nc.vector.memset(ones128, 1.0)
            ones_row = const.tile([1, 128], BF16)
            nc.vector.memset(ones_row, 1.0)
            ea_ring = const.tile([128, 2, EARING, 2, SP1], BF16)
            bout_bc = const.tile([128, DIM], F32)
            bout_bf = const.tile([1, DIM], BF16)

            def emit_late_consts():
                nc.sync.dma_start(out=wout_sb, in_=wout[:, :].rearrange("(c p) o -> p c o", p=128))
                nc.sync.dma_start(out=bout_bc, in_=bout[:, :].to_broadcast([128, DIM]))
                nc.vector.tensor_copy(bout_bf, bout_bc[0:1, :])

            S = [dict() for _ in range(NB)]

            def alloc_batch(b):
                st = S[b]
                st["xT"] = perb.tile([128, KC, N], BF16, tag="xt", name=f"xt{b}")
                st["qT"] = perb.tile([128, 4, F, SP1], BF16, tag="qT", name=f"qT{b}")
                st["kT"] = perb.tile([128, 4, F, SP1], BF16, tag="kT", name=f"kT{b}")
                st["vfl"] = vflat_pool.tile([128, NT, INNER], BF16, tag="vfl", name=f"vfl{b}")
                st["va"] = perb.tile([128, F, INNER], BF16, tag="vfra", name=f"vfra{b}")
                st["vb"] = perb.tile([128, F, INNER], BF16, tag="vfrb", name=f"vfrb{b}")
                st["attnT"] = perb.tile([128, KC, N], BF16, tag="attnT", name=f"attnT{b}")
                oscls = clsp.tile([128, 2, 4], F32, tag="oscls", name=f"oscls{b}")
                st["oscls"] = oscls
                st["ocls"] = oscls[:, 0, :]
                st["scls"] = oscls[:, 1, :]
                st["ecc_row"] = clsp.tile([1, H], BF16, tag="eccrow", name=f"eccrow{b}")
                st["vTcls"] = clsp.tile([128, 4], F32, tag="vTcls", name=f"vTcls{b}")
                st["ecc_bc"] = clsp.tile([128, 4], F32, tag="eccbc", name=f"eccbc{b}")
                st["rcls"] = clsp.tile([128, 4], F32, tag="rcls", name=f"rcls{b}")
                st["tevc"] = clsp.tile([128, 4], F32, tag="tevc", name=f"tevc{b}")
                st["tcorr"] = clsp.tile([128, 4], F32, tag="tcorr", name=f"tcorr{b}")
                nc.vector.memset(oscls, 0.0)

            def emit_xT_bundle(b, ci, eng):
                (t0, tl) = TCH[ci]
                for kc in range(KC):
                    eng.dma_start(
                        out=S[b]["xT"][:, kc, t0:t0 + tl],
                        in_=xT[b, 128 * kc:128 * (kc + 1), t0:t0 + tl])

            def emit_qk_item(b, oc, ci):
                isq = oc < 4
                hp = oc if isq else oc - 4
                (t0, tl) = TCH[ci]
                ps = ps_proj.tile([128, 512], F32, tag="proj")
                for kc in range(KC):
                    nc.tensor.matmul(
                        ps[:, :tl],
                        lhsT=wqkv_sb[:, kc, oc * 128:(oc + 1) * 128],
                        rhs=S[b]["xT"][:, kc, t0:t0 + tl],
                        start=(kc == 0),
                        stop=(kc == KC - 1),
                    )
                dst = S[b]["qT"] if isq else S[b]["kT"]
                eng = nc.scalar if isq else nc.vector
                cp = eng.copy if isq else eng.tensor_copy
                o0 = 0 if isq else 1
                if ci == 0:
                    cp(
                        dst[:, hp, 0:2, o0:o0 + SP],
                        ps[:, 1:tl].rearrange("p (a s) -> p a s", a=2),
                    )
                    ccol = SP if isq else 0
                    cp(
                        dst[:, hp, 0:F, ccol:ccol + 1],
                        ps[:, None, 0:1].to_broadcast([128, F, 1]),
                    )
                else:
                    cp(
                        dst[:, hp, 2 * ci:2 * ci + 2, o0:o0 + SP],
                        ps[:, :tl].rearrange("p (a s) -> p a s", a=2),
                    )

            def emit_v_item(b, t):
                m = 128 if t < NT - 1 else LAST
                vfl = S[b]["vfl"]
                ps = ps_proj.tile([128, 512], F32, tag="proj")
                for kc in range(KC):
                    nc.tensor.matmul(
                        ps[:m, :],
                        lhsT=S[b]["xT"][:, kc, 128 * t:128 * t + m],
                        rhs=wqkv_sb[:, kc, 2 * INNER:3 * INNER],
                        start=(kc == 0),
                        stop=(kc == KC - 1),
                    )
                if t >= 4 and t % 2 == 0:
                    nc.scalar.copy(vfl[:m, t, :], ps[:m, :])
                else:
                    nc.vector.tensor_copy(vfl[:m, t, :], ps[:m, :])
                # frame-aligned rearrangement pieces sourced from this tile
                for (kind, f, p0, d0, l) in rearr.get(t, []):
                    dst = S[b]["va"] if kind == "a" else S[b]["vb"]
                    nc.sync.dma_start(out=dst[d0:d0 + l, f, :], in_=vfl[p0:p0 + l, t, :])
                if t == 0:
                    nc.sync.dma_start(
                        out=S[b]["va"][0:1, 0:F, :],
                        in_=vfl[0:1, 0, None, :].to_broadcast([1, F, INNER]),
                    )
                    pvt = ps_proj.tile([128, 512], F32, tag="proj")
                    for hp in range(4):
                        nc.tensor.matmul(
                            pvt[:, hp:hp + 1],
                            lhsT=vfl[0:1, 0, 128 * hp:128 * (hp + 1)],
                            rhs=ones_row[0:1, 0:1],
                            start=True, stop=True,
                        )
                    nc.vector.tensor_copy(S[b]["vTcls"], pvt[:, 0:4])

            def pass1(g, b, hp, f):
                st_t = ps_st.tile([128, 2, 2, 256], F32, tag="st")
                ea = ea_ring[:, :, g % EARING, :, :]
                qT, kT = S[b]["qT"], S[b]["kT"]
                for par in range(2):
                    rows = slice(64 * par, 64 * par + 64)
                    nc.tensor.matmul(
                        st_t[:, par, 0, 0:SP1],
                        lhsT=kT[rows, hp, f, 0:128],
                        rhs=qT[rows, hp, f, :],
                        start=True, stop=True,
                    )
                for par in range(2):
                    rows = slice(64 * par, 64 * par + 64)
                    nc.tensor.matmul(
                        st_t[0:69, par, 1, 0:SP1],
                        lhsT=kT[rows, hp, f, 128:SP1],
                        rhs=qT[rows, hp, f, :],
                        start=True, stop=True,
                    )
                nc.scalar.activation(ea, st_t[:, :, :, 0:SP1], EXP)
                if f == 0:
                    for par in range(2):
                        h = 2 * hp + par
                        nc.scalar.copy(S[b]["ecc_row"][0:1, h:h + 1],
                                       ea[0:1, par, 0, SP:SP1])

            def pass2(g, b, hp, f):
                ea = ea_ring[:, :, g % EARING, :, :]
                va, vb = S[b]["va"], S[b]["vb"]
                po = ps_po.tile([128, 512], F32, tag="po")
                for par in range(2):
                    rows = slice(64 * par, 64 * par + 64)
                    hs = slice(DH * (2 * hp + par), DH * (2 * hp + par + 1))
                    nc.tensor.matmul(
                        po[rows, 0:SP1],
                        lhsT=va[:, f, hs],
                        rhs=ea[:, par, 0, :],
                        start=True, stop=False,
                    )
                    nc.tensor.matmul(
                        po[rows, 0:SP1],
                        lhsT=vb[0:69, f, hs],
                        rhs=ea[0:69, par, 1, :],
                        start=False, stop=True,
                    )
                for par in range(2):
                    rows = slice(64 * par, 64 * par + 64)
                    nc.tensor.matmul(
                        po[rows, 256:256 + SP1],
                        lhsT=ones128[:, 0:64],
                        rhs=ea[:, par, 0, :],
                        start=True, stop=False,
                    )
                    nc.tensor.matmul(
                        po[rows, 256:256 + SP1],
                        lhsT=ones128[0:69, 0:64],
                        rhs=ea[0:69, par, 1, :],
                        start=False, stop=True,
                    )
                rbc = small.tile([128, SP1], F32, tag="rbc")
                nc.vector.reciprocal_approx_fast(rbc, po[:, 256:256 + SP1])
                nc.vector.tensor_tensor(
                    S[b]["attnT"][:, hp, 1 + SP * f:1 + SP * (f + 1)],
                    po[:, 0:SP], rbc[:, 0:SP], MULT,
                )
                nc.vector.tensor_tensor(
                    S[b]["oscls"][:, :, hp], po[:, SP:SP + 257:256],
                    S[b]["oscls"][:, :, hp], ADD,
                )

            def cls_finalize(b):
                st = S[b]
                pec = ps_proj.tile([128, 512], F32, tag="proj")
                for hp in range(4):
                    for par in range(2):
                        h = 2 * hp + par
                        rows = slice(64 * par, 64 * par + 64)
                        nc.tensor.matmul(pec[rows, hp:hp + 1],
                                         lhsT=ones_row[0:1, 0:64],
                                         rhs=st["ecc_row"][0:1, h:h + 1],
                                         start=True, stop=True)
                nc.vector.tensor_copy(st["ecc_bc"], pec[:, 0:4])
                nc.vector.scalar_tensor_tensor(
                    st["scls"], st["ecc_bc"], -7.0, st["scls"], op0=MULT, op1=ADD,
                )
                nc.vector.reciprocal_approx_fast(st["rcls"], st["scls"])
                nc.vector.tensor_tensor(st["tevc"], st["ecc_bc"], st["vTcls"], MULT)
                nc.vector.scalar_tensor_tensor(
                    st["tcorr"], st["tevc"], -7.0, st["ocls"], op0=MULT, op1=ADD,
                )
                nc.vector.tensor_tensor(st["tcorr"], st["tcorr"], st["rcls"], MULT)
                nc.vector.tensor_copy(st["attnT"][:, 0:4, 0:1], st["tcorr"][:, :, None])

            def emit_outproj(b, t):
                m = 128 if t < NT - 1 else LAST
                ps = ps_proj.tile([128, 512], F32, tag="proj")
                act_tail = b == 1 and t in (10, 12, 0)
                for kc in range(KC):
                    nc.tensor.matmul(
                        ps[:m, :],
                        lhsT=S[b]["attnT"][:, kc, 128 * t:128 * t + m],
                        rhs=wout_sb[:, kc, :],
                        start=(kc == 0),
                        stop=(kc == KC - 1) and not act_tail,
                    )
                osb = outp.tile([128, DIM], F32, tag="out")
                if act_tail:
                    nc.tensor.matmul(
                        ps[:m, :],
                        lhsT=ones_row[0:1, 0:m],
                        rhs=bout_bf[0:1, :],
                        start=False, stop=True,
                    )
                    nc.scalar.copy(osb[:m, :], ps[:m, :])
                else:
                    nc.vector.tensor_tensor(osb[:m, :], ps[:m, :], bout_bc[:m, :], ADD)
                nc.sync.dma_start(out=out[b, 128 * t:128 * t + m, :], in_=osb[:m, :])

            # ---------------- emission schedule ----------------
            sched = {}

            def add(g, fn, *a):
                sched.setdefault(g, []).append((fn, a))

            # feed: b0 qk for hp 1..3 -> iterations 0..11, 2 items/step
            g = 0
            cnt = 0
            for hp in (1, 2, 3):
                for ci in range(4):
                    for oc in (hp, hp + 4):
                        add(cnt // 2, emit_qk_item, 0, oc, ci)
                        cnt += 1
            # b1: alloc + xT bundles on sync
            add(2, alloc_batch, 1)
            for ci in range(4):
                add(2 + 2 * ci, emit_xT_bundle, 1, ci, nc.sync)
            # b1 V items
            for t in range(NT):
                add(8 + t, emit_v_item, 1, t)
            # b1 qk, hp-major
            cnt = 0
            for hp in range(4):
                for ci in range(4):
                    for oc in (hp, hp + 4):
                        add(14 + cnt // 2, emit_qk_item, 1, oc, ci)
                        cnt += 1
            # b0 outproj (b0 attention done at pass2 of step 31 -> iteration 33)
            add(35, cls_finalize, 0)
            for t in range(1, NT):
                add(36 + t - 1, emit_outproj, 0, t)
            add(48, emit_outproj, 0, 0)
            # b1 outproj as frames complete (b1 attention frame-major, steps 32..63)
            add(NSTEP + LAG, cls_finalize, 1)
            for t in range(1, NT):
                fmax = min(7, (128 * (t + 1) - 2) // SP)
                add(min(NSTEP + LAG, 40 + 4 * fmax), emit_outproj, 1, t)
            add(NSTEP + LAG, emit_outproj, 1, 0)

            # ---- stage A: b0 projection, V first for early attention start.
            # Critical-path loads are fine-grained and spread over the scalar,
            # sync and gpsimd DMA queues so the first V-proj group starts ASAP.
            alloc_batch(0)
            for kc in range(KC):
                nc.scalar.dma_start(out=S[0]["xT"][:, kc, 0:1 + SP],
                                    in_=xT[0, 128 * kc:128 * (kc + 1), 0:1 + SP])
            for kc in range(KC):
                nc.scalar.dma_start(out=S[0]["xT"][:, kc, 1 + SP:1 + 2 * SP],
                                    in_=xT[0, 128 * kc:128 * (kc + 1), 1 + SP:1 + 2 * SP])
            emit_xT_bundle(0, 1, nc.scalar)
            nc.sync.dma_start(out=wqkv_sb[:, :, 0:INNER],
                              in_=wqkv[:, 0:INNER].rearrange("(c p) o -> p c o", p=128))
            nc.sync.dma_start(out=wqkv_sb[:, :, INNER:2 * INNER],
                              in_=wqkv[:, INNER:2 * INNER].rearrange("(c p) o -> p c o", p=128))
            emit_xT_bundle(0, 2, nc.sync)
            emit_xT_bundle(0, 3, nc.sync)
            for t in range(6):
                emit_v_item(0, t)
            for ci in range(4):
                emit_qk_item(0, 0, ci)
                emit_qk_item(0, 4, ci)
            for t in range(6, NT):
                emit_v_item(0, t)
            add(20, emit_late_consts)

            # ---- main interleaved loop
            # b0 hp-major (projection feeds hp by hp); b1 frame-major (lets
            # out-projection start as frames complete)
            steps = [(0, hp, f) for hp in range(4) for f in range(F)]
            steps += [(1, hp, f) for f in range(F) for hp in range(4)]
            for g in range(NSTEP + LAG + 1):
                items = sched.pop(g, [])
                if items:
                    fn, a = items.pop(0)
                    fn(*a)
                if g < NSTEP:
                    pass1(g, *steps[g])
                if g >= LAG and g - LAG < NSTEP:
                    pass2(g - LAG, *steps[g - LAG])
                for (fn, a) in items:
                    fn(*a)
            for g in sorted(sched):
                for (fn, a) in sched[g]:
                    fn(*a)

    nc.finalize()
    return nc


_CACHE = {}


def _get_nc():
    if "nc" not in _CACHE:
        _CACHE["nc"] = build_nc()
    return _CACHE["nc"]


def prepare_in_maps(x, f, W_qkv, W_out, b_out):
    assert int(f) == F
    x = np.asarray(x, dtype=np.float32)
    W_qkv = np.asarray(W_qkv, dtype=np.float32).copy()
    W_out = np.asarray(W_out, dtype=np.float32)
    b_out = np.asarray(b_out, dtype=np.float32)
    W_qkv[:, :INNER] *= DH ** -0.5
    wqkv_bf = W_qkv.astype(NPBF)
    wout_bf = W_out.astype(NPBF)
    bout_np = b_out.reshape(1, DIM)
    xT = np.ascontiguousarray(x.transpose(0, 2, 1)).astype(NPBF)
    in_maps = []
    for c in range(NCORES):
        in_maps.append({
            "xT": np.ascontiguousarray(xT[NB * c:NB * (c + 1)]),
            "wqkv": wqkv_bf,
            "wout": wout_bf,
            "bout": bout_np,
        })
    return in_maps


def kernel(x, f, W_qkv, W_out, b_out):
    nc = _get_nc()
    in_maps = prepare_in_maps(x, f, W_qkv, W_out, b_out)
    res = bass_utils.run_bass_kernel_spmd(nc, in_maps, list(range(NCORES)))
    return np.concatenate([r["out"] for r in res.results], axis=0)


# revision 16
# speedup vs baseline: 1.0374x; 1.0374x over previous
"""DividedAttention (TimeSformer-style divided space-time attention) on 8 trn2 cores.

Sharding: pure data-parallel over batch B=16 -> 2 batch items per core.

v2: two-batch software pipeline. The per-core schedule interleaves batch 1's
QKV projection into batch 0's attention steps and batch 0's out-projection
into batch 1's attention steps, so the PE never idles long enough for the
HAM clock gate to re-throttle it to 1.2 GHz (the dominant cost in v1).
Other changes vs v1:
  - one fused exp per attention step (st psum tile [128,2par,2chunk,256],
    single ACTIVATE over all 4 S^T quadrants)
  - PSUM re-plan: proj 2 banks + st 2x2 banks + po 2x1 bank = 8
  - AV output and column sums share one psum bank (cols 0:197 / 256:453)
  - bias + psum->sbuf staging fused in one vector tensor_tensor; the three
    chain-bound final tiles instead use scalar-engine copy with the bias
    applied by a K=1 ones-row matmul, so the vector engine is not the
    serial tail
  - exp(S^T) results live in a single 4-slot ring tile indexed by step,
    enabling frame-major step order for batch 1 (out-projection tiles
    start as frames complete)
  - xT loads split across the scalar and sync hwdge queues
"""
import sys

sys.path.insert(0, "/opt/trn_rl_repo")

import numpy as np
import ml_dtypes

from concourse import bacc
import concourse.mybir as mybir
import concourse.tile as tile
from concourse import bass_utils

BF16 = mybir.dt.bfloat16
F32 = mybir.dt.float32
NPBF = ml_dtypes.bfloat16

B, SP, F, DIM, H, DH = 16, 196, 8, 512, 8, 64
INNER = H * DH            # 512
N = 1 + F * SP            # 1569
SP1 = SP + 1              # 197
NCORES = 8
NB = B // NCORES          # 2
KC = DIM // 128           # 4
NT = (N + 127) // 128     # 13
LAST = N - 128 * (NT - 1)  # 33
TCH = [(0, 1 + 2 * SP), (1 + 2 * SP, 2 * SP), (1 + 4 * SP, 2 * SP), (1 + 6 * SP, 2 * SP)]

EXP = mybir.ActivationFunctionType.Exp
ADD = mybir.AluOpType.add
MULT = mybir.AluOpType.mult

LAG = 2
EARING = 4
NSTEP = NB * 32           # 64 attention steps, hp-major per batch


def _v_pieces(tok0, length):
    out = []
    done = 0
    while done < length:
        tok = tok0 + done
        t, p0 = divmod(tok, 128)
        l = min(128 - p0, length - done)
        out.append((t, p0, done, l))
        done += l
    return out


def _rearr_by_tile():
    """v_flat tile -> list of (dst_kind, frame, src_p0, dst_p0, len)."""
    by_tile = {}
    for f in range(F):
        for (t, p0, d0, l) in _v_pieces(1 + SP * f, 127):
            by_tile.setdefault(t, []).append(("a", f, p0, 1 + d0, l))
        for (t, p0, d0, l) in _v_pieces(128 + SP * f, 69):
            by_tile.setdefault(t, []).append(("b", f, p0, d0, l))
    return by_tile


def build_nc():
    nc = bacc.Bacc(num_devices=NCORES)

    xT = nc.declare_dram_parameter("xT", [NB, DIM, N], BF16, isOutput=False)
    wqkv = nc.declare_dram_parameter("wqkv", [DIM, 3 * INNER], BF16, isOutput=False)
    wout = nc.declare_dram_parameter("wout", [INNER, DIM], BF16, isOutput=False)
    bout = nc.declare_dram_parameter("bout", [1, DIM], F32, isOutput=False)
    out = nc.declare_dram_parameter("out", [NB, N, DIM], F32, isOutput=True)

    rearr = _rearr_by_tile()

    with tile.TileContext(nc) as tc:
        with (
            tc.tile_pool(name="const", bufs=1) as const,
            tc.tile_pool(name="perb", bufs=2) as perb,
            tc.tile_pool(name="vflat", bufs=1) as vflat_pool,
            tc.tile_pool(name="clsp", bufs=2) as clsp,
            tc.tile_pool(name="small", bufs=3) as small,
            tc.tile_pool(name="outp", bufs=3) as outp,
            tc.tile_pool(name="ps_proj", bufs=2, space="PSUM") as ps_proj,
            tc.tile_pool(name="ps_st", bufs=2, space="PSUM") as ps_st,
            tc.tile_pool(name="ps_po", bufs=2, space="PSUM") as ps_po,
        ):
            # ---- constants
            wqkv_sb = const.tile([128, KC, 3 * INNER], BF16)
            nc.sync.dma_start(out=wqkv_sb[:, :, 2 * INNER:3 * INNER],
                              in_=wqkv[:, 2 * INNER:3 * INNER].rearrange("(c p) o -> p c o", p=128))
            wout_sb = const.tile([128, KC, DIM], BF16)
            ones128 = const.tile([128, 64], BF16)
            nc.vector.memset(ones128, 1.0)
            ones_row = const.tile([1, 128], BF16)
            nc.vector.memset(ones_row, 1.0)
            ea_ring = const.tile([128, 2, EARING, 2, SP1], BF16)
            bout_bc = const.tile([128, DIM], F32)
            nc.sync.dma_start(out=bout_bc, in_=bout[:, :].to_broadcast([128, DIM]))
            bout_bf = const.tile([1, DIM], BF16)
            nc.vector.tensor_copy(bout_bf, bout_bc[0:1, :])

            S = [dict() for _ in range(NB)]

            def alloc_batch(b):
                st = S[b]
                st["xT"] = perb.tile([128, KC, N], BF16, tag="xt", name=f"xt{b}")
                st["qT"] = perb.tile([128, 4, F, SP1], BF16, tag="qT", name=f"qT{b}")
                st["kT"] = perb.tile([128, 4, F, SP1], BF16, tag="kT", name=f"kT{b}")
                st["vfl"] = vflat_pool.tile([128, NT, INNER], BF16, tag="vfl", name=f"vfl{b}")
                st["va"] = perb.tile([128, F, INNER], BF16, tag="vfra", name=f"vfra{b}")
                st["vb"] = perb.tile([128, F, INNER], BF16, tag="vfrb", name=f"vfrb{b}")
                st["attnT"] = perb.tile([128, KC, N], BF16, tag="attnT", name=f"attnT{b}")
                oscls = clsp.tile([128, 2, 4], F32, tag="oscls", name=f"oscls{b}")
                st["oscls"] = oscls
                st["ocls"] = oscls[:, 0, :]
                st["scls"] = oscls[:, 1, :]
                st["ecc_row"] = clsp.tile([1, H], BF16, tag="eccrow", name=f"eccrow{b}")
                st["vTcls"] = clsp.tile([128, 4], F32, tag="vTcls", name=f"vTcls{b}")
                st["ecc_bc"] = clsp.tile([128, 4], F32, tag="eccbc", name=f"eccbc{b}")
                st["rcls"] = clsp.tile([128, 4], F32, tag="rcls", name=f"rcls{b}")
                st["tevc"] = clsp.tile([128, 4], F32, tag="tevc", name=f"tevc{b}")
                st["tcorr"] = clsp.tile([128, 4], F32, tag="tcorr", name=f"tcorr{b}")
                nc.vector.memset(oscls, 0.0)

            def emit_xT_bundle(b, ci, eng):
                (t0, tl) = TCH[ci]
                for kc in range(KC):
                    eng.dma_start(
                        out=S[b]["xT"][:, kc, t0:t0 + tl],
                        in_=xT[b, 128 * kc:128 * (kc + 1), t0:t0 + tl])

            def emit_qk_item(b, oc, ci):
                isq = oc < 4
                hp = oc if isq else oc - 4
                (t0, tl) = TCH[ci]
                ps = ps_proj.tile([128, 512], F32, tag="proj")
                for kc in range(KC):
                    nc.tensor.matmul(
                        ps[:, :tl],
                        lhsT=wqkv_sb[:, kc, oc * 128:(oc + 1) * 128],
                        rhs=S[b]["xT"][:, kc, t0:t0 + tl],
                        start=(kc == 0),
                        stop=(kc == KC - 1),
                    )
                dst = S[b]["qT"] if isq else S[b]["kT"]
                eng = nc.scalar if isq else nc.vector
                cp = eng.copy if isq else eng.tensor_copy
                o0 = 0 if isq else 1
                if ci == 0:
                    cp(
                        dst[:, hp, 0:2, o0:o0 + SP],
                        ps[:, 1:tl].rearrange("p (a s) -> p a s", a=2),
                    )
                    ccol = SP if isq else 0
                    cp(
                        dst[:, hp, 0:F, ccol:ccol + 1],
                        ps[:, None, 0:1].to_broadcast([128, F, 1]),
                    )
                else:
                    cp(
                        dst[:, hp, 2 * ci:2 * ci + 2, o0:o0 + SP],
                        ps[:, :tl].rearrange("p (a s) -> p a s", a=2),
                    )

            def emit_v_item(b, t):
                m = 128 if t < NT - 1 else LAST
                vfl = S[b]["vfl"]
                ps = ps_proj.tile([128, 512], F32, tag="proj")
                for kc in range(KC):
                    nc.tensor.matmul(
                        ps[:m, :],
                        lhsT=S[b]["xT"][:, kc, 128 * t:128 * t + m],
                        rhs=wqkv_sb[:, kc, 2 * INNER:3 * INNER],
                        start=(kc == 0),
                        stop=(kc == KC - 1),
                    )
                if t % 2 == 0:
                    nc.scalar.copy(vfl[:m, t, :], ps[:m, :])
                else:
                    nc.vector.tensor_copy(vfl[:m, t, :], ps[:m, :])
                # frame-aligned rearrangement pieces sourced from this tile
                for (kind, f, p0, d0, l) in rearr.get(t, []):
                    dst = S[b]["va"] if kind == "a" else S[b]["vb"]
                    nc.sync.dma_start(out=dst[d0:d0 + l, f, :], in_=vfl[p0:p0 + l, t, :])
                if t == 0:
                    nc.sync.dma_start(
                        out=S[b]["va"][0:1, 0:F, :],
                        in_=vfl[0:1, 0, None, :].to_broadcast([1, F, INNER]),
                    )
                    pvt = ps_proj.tile([128, 512], F32, tag="proj")
                    for hp in range(4):
                        nc.tensor.matmul(
                            pvt[:, hp:hp + 1],
                            lhsT=vfl[0:1, 0, 128 * hp:128 * (hp + 1)],
                            rhs=ones_row[0:1, 0:1],
                            start=True, stop=True,
                        )
                    nc.vector.tensor_copy(S[b]["vTcls"], pvt[:, 0:4])

            def pass1(g, b, hp, f):
                st_t = ps_st.tile([128, 2, 2, 256], F32, tag="st")
                ea = ea_ring[:, :, g % EARING, :, :]
                qT, kT = S[b]["qT"], S[b]["kT"]
                for par in range(2):
                    rows = slice(64 * par, 64 * par + 64)
                    nc.tensor.matmul(
                        st_t[:, par, 0, 0:SP1],
                        lhsT=kT[rows, hp, f, 0:128],
                        rhs=qT[rows, hp, f, :],
                        start=True, stop=True,
                    )
                for par in range(2):
                    rows = slice(64 * par, 64 * par + 64)
                    nc.tensor.matmul(
                        st_t[0:69, par, 1, 0:SP1],
                        lhsT=kT[rows, hp, f, 128:SP1],
                        rhs=qT[rows, hp, f, :],
                        start=True, stop=True,
                    )
                nc.scalar.activation(ea, st_t[:, :, :, 0:SP1], EXP)
                if f == 0:
                    for par in range(2):
                        h = 2 * hp + par
                        nc.scalar.copy(S[b]["ecc_row"][0:1, h:h + 1],
                                       ea[0:1, par, 0, SP:SP1])

            def pass2(g, b, hp, f):
                ea = ea_ring[:, :, g % EARING, :, :]
                va, vb = S[b]["va"], S[b]["vb"]
                po = ps_po.tile([128, 512], F32, tag="po")
                for par in range(2):
                    rows = slice(64 * par, 64 * par + 64)
                    hs = slice(DH * (2 * hp + par), DH * (2 * hp + par + 1))
                    nc.tensor.matmul(
                        po[rows, 0:SP1],
                        lhsT=va[:, f, hs],
                        rhs=ea[:, par, 0, :],
                        start=True, stop=False,
                    )
                    nc.tensor.matmul(
                        po[rows, 0:SP1],
                        lhsT=vb[0:69, f, hs],
                        rhs=ea[0:69, par, 1, :],
                        start=False, stop=True,
                    )
                for par in range(2):
                    rows = slice(64 * par, 64 * par + 64)
                    nc.tensor.matmul(
                        po[rows, 256:256 + SP1],
                        lhsT=ones128[:, 0:64],
                        rhs=ea[:, par, 0, :],
                        start=True, stop=False,
                    )
                    nc.tensor.matmul(
                        po[rows, 256:256 + SP1],
                        lhsT=ones128[0:69, 0:64],
                        rhs=ea[0:69, par, 1, :],
                        start=False, stop=True,
                    )
                rbc = small.tile([128, SP1], F32, tag="rbc")
                nc.vector.reciprocal_approx_fast(rbc, po[:, 256:256 + SP1])
                nc.vector.tensor_tensor(
                    S[b]["attnT"][:, hp, 1 + SP * f:1 + SP * (f + 1)],
                    po[:, 0:SP], rbc[:, 0:SP], MULT,
                )
                nc.vector.tensor_tensor(
                    S[b]["oscls"][:, :, hp], po[:, SP:SP + 257:256],
                    S[b]["oscls"][:, :, hp], ADD,
                )

            def cls_finalize(b):
                st = S[b]
                pec = ps_proj.tile([128, 512], F32, tag="proj")
                for hp in range(4):
                    for par in range(2):
                        h = 2 * hp + par
                        rows = slice(64 * par, 64 * par + 64)
                        nc.tensor.matmul(pec[rows, hp:hp + 1],
                                         lhsT=ones_row[0:1, 0:64],
                                         rhs=st["ecc_row"][0:1, h:h + 1],
                                         start=True, stop=True)
                nc.vector.tensor_copy(st["ecc_bc"], pec[:, 0:4])
                nc.vector.scalar_tensor_tensor(
                    st["scls"], st["ecc_bc"], -7.0, st["scls"], op0=MULT, op1=ADD,
                )
                nc.vector.reciprocal_approx_fast(st["rcls"], st["scls"])
                nc.vector.tensor_tensor(st["tevc"], st["ecc_bc"], st["vTcls"], MULT)
                nc.vector.scalar_tensor_tensor(
                    st["tcorr"], st["tevc"], -7.0, st["ocls"], op0=MULT, op1=ADD,
                )
                nc.vector.tensor_tensor(st["tcorr"], st["tcorr"], st["rcls"], MULT)
                nc.vector.tensor_copy(st["attnT"][:, 0:4, 0:1], st["tcorr"][:, :, None])

            def emit_outproj(b, t):
                m = 128 if t < NT - 1 else LAST
                ps = ps_proj.tile([128, 512], F32, tag="proj")
                act_tail = b == 1 and t in (10, 12, 0)
                for kc in range(KC):
                    nc.tensor.matmul(
                        ps[:m, :],
                        lhsT=S[b]["attnT"][:, kc, 128 * t:128 * t + m],
                        rhs=wout_sb[:, kc, :],
                        start=(kc == 0),
                        stop=(kc == KC - 1) and not act_tail,
                    )
                osb = outp.tile([128, DIM], F32, tag="out")
                if act_tail:
                    nc.tensor.matmul(
                        ps[:m, :],
                        lhsT=ones_row[0:1, 0:m],
                        rhs=bout_bf[0:1, :],
                        start=False, stop=True,
                    )
                    nc.scalar.copy(osb[:m, :], ps[:m, :])
                else:
                    nc.vector.tensor_tensor(osb[:m, :], ps[:m, :], bout_bc[:m, :], ADD)
                nc.sync.dma_start(out=out[b, 128 * t:128 * t + m, :], in_=osb[:m, :])

            # ---------------- emission schedule ----------------
            sched = {}

            def add(g, fn, *a):
                sched.setdefault(g, []).append((fn, a))

            # feed: b0 qk for hp 1..3 -> iterations 0..11, 2 items/step
            g = 0
            cnt = 0
            for hp in (1, 2, 3):
                for ci in range(4):
                    for oc in (hp, hp + 4):
                        add(cnt // 2, emit_qk_item, 0, oc, ci)
                        cnt += 1
            # b1: alloc + xT bundles on sync
            add(2, alloc_batch, 1)
            for ci in range(4):
                add(2 + 2 * ci, emit_xT_bundle, 1, ci, nc.sync)
            # b1 V items
            for t in range(NT):
                add(8 + t, emit_v_item, 1, t)
            # b1 qk, hp-major
            cnt = 0
            for hp in range(4):
                for ci in range(4):
                    for oc in (hp, hp + 4):
                        add(14 + cnt // 2, emit_qk_item, 1, oc, ci)
                        cnt += 1
            # b0 outproj (b0 attention done at pass2 of step 31 -> iteration 33)
            add(35, cls_finalize, 0)
            for t in range(1, NT):
                add(36 + t - 1, emit_outproj, 0, t)
            add(48, emit_outproj, 0, 0)
            # b1 outproj as frames complete (b1 attention frame-major, steps 32..63)
            add(NSTEP + LAG, cls_finalize, 1)
            for t in range(1, NT):
                fmax = min(7, (128 * (t + 1) - 2) // SP)
                add(min(NSTEP + LAG, 40 + 4 * fmax), emit_outproj, 1, t)
            add(NSTEP + LAG, emit_outproj, 1, 0)

            # ---- stage A: b0 projection, V first for early attention start
            alloc_batch(0)
            emit_xT_bundle(0, 0, nc.scalar)
            emit_xT_bundle(0, 1, nc.scalar)
            emit_xT_bundle(0, 2, nc.sync)
            emit_xT_bundle(0, 3, nc.sync)
            nc.sync.dma_start(out=wqkv_sb[:, :, 0:2 * INNER],
                              in_=wqkv[:, 0:2 * INNER].rearrange("(c p) o -> p c o", p=128))
            nc.sync.dma_start(out=wout_sb, in_=wout[:, :].rearrange("(c p) o -> p c o", p=128))
            for t in range(6):
                emit_v_item(0, t)
            for ci in range(4):
                emit_qk_item(0, 0, ci)
                emit_qk_item(0, 4, ci)
            for t in range(6, NT):
                emit_v_item(0, t)

            # ---- main interleaved loop
            # b0 hp-major (projection feeds hp by hp); b1 frame-major (lets
            # out-projection start as frames complete)
            steps = [(0, hp, f) for hp in range(4) for f in range(F)]
            steps += [(1, hp, f) for f in range(F) for hp in range(4)]
            for g in range(NSTEP + LAG + 1):
                items = sched.pop(g, [])
                if items:
                    fn, a = items.pop(0)
                    fn(*a)
                if g < NSTEP:
                    pass1(g, *steps[g])
                if g >= LAG and g - LAG < NSTEP:
                    pass2(g - LAG, *steps[g - LAG])
                for (fn, a) in items:
                    fn(*a)
            for g in sorted(sched):
                for (fn, a) in sched[g]:
                    fn(*a)

    nc.finalize()
    return nc


_CACHE = {}


def _get_nc():
    if "nc" not in _CACHE:
        _CACHE["nc"] = build_nc()
    return _CACHE["nc"]


def prepare_in_maps(x, f, W_qkv, W_out, b_out):
    assert int(f) == F
    x = np.asarray(x, dtype=np.float32)
    W_qkv = np.asarray(W_qkv, dtype=np.float32).copy()
    W_out = np.asarray(W_out, dtype=np.float32)
    b_out = np.asarray(b_out, dtype=np.float32)
    W_qkv[:, :INNER] *= DH ** -0.5
    wqkv_bf = W_qkv.astype(NPBF)
    wout_bf = W_out.astype(NPBF)
    bout_np = b_out.reshape(1, DIM)
    xT = np.ascontiguousarray(x.transpose(0, 2, 1)).astype(NPBF)
    in_maps = []
    for c in range(NCORES):
        in_maps.append({
            "xT": np.ascontiguousarray(xT[NB * c:NB * (c + 1)]),
            "wqkv": wqkv_bf,
            "wout": wout_bf,
            "bout": bout_np,
        })
    return in_maps


def kernel(x, f, W_qkv, W_out, b_out):
    nc = _get_nc()
    in_maps = prepare_in_maps(x, f, W_qkv, W_out, b_out)
    res = bass_utils.run_bass_kernel_spmd(nc, in_maps, list(range(NCORES)))
    return np.concatenate([r["out"] for r in res.results], axis=0)
